# revision 33
# baseline (speedup 1.0000x reference)
"""CGCNNConv on 8 TRN2 NeuronCores — transfer-optimized.

The axon tunnel (~63 MB/s effective) dominates wall time, so the design
minimizes host<->device bytes:
  - nbr_fea ships as fp8(e4m3), feature-major [NT, M, 64, TW] (9.8MB/core).
  - atom features ship SHARDED: each core sends its [12501, 128] bf16
    zero-padded slice (cols 64:128 zero, last row zero); a device AllGather
    builds the full [100008, 128] bf16 table once.  Row of atom j is
    r = j + j//12500 (one zero row after each shard); chunk c = r//25002
    gives 4 int16-addressable gather chunks, each with zero rows at local
    12500 and 25001 (used as the "no contribution" target).
  - neighbor gathers run ON DEVICE via gpsimd dma_gather (transpose mode,
    256B rows -> features land partition-major), 4 chunk-gathers per tile
    (6656 idxs = 12*512 edge slots + 512 self slots), summed with 3 DVE adds.
  - output is just delta = softplus(bn2(msg)) as [64, NPAD] bf16; the final
    atom_fea + delta add happens on host in f32.
Device per core, single NEFF:
  pass1: per t: 4 dma_gathers + adds -> gathered/self features; per (t,m):
         3 K=64 matmuls (self, gathered, nbr) -> PSUM [128, 512];
         DVE copies y->bf16 (+sum(y)), ACT accumulates sum(y^2),
         y stored to DRAM scratch [128, 153600] bf16.
  AllReduce [128, 2] -> BN1 scale/shift (linear bias b folds out).
  pass2: reload y halves stacked, ACT sigmoid/softplus, DVE -> msg [64, NPAD].
  AllReduce [64, 2] -> BN2; pass3: softplus(bn2(msg)) -> bf16 delta out.
"""
import os
import sys
sys.path.insert(0, '/opt/trn_rl_repo')
import numpy as np
import ml_dtypes

from concourse import bass, mybir
from concourse import library_config
from concourse.tile import TileContext
from concourse.bass_utils import run_bass_kernel_spmd

NCORES = 8
N = 100000
M = 12
F = 64                      # atom/nbr feature len
OUT = 128                   # 2F
NLOC = 12500                # atoms per core
NPAD = 12800                # padded atoms per core
NT = 25                     # tiles of 512 atoms
TW = 512                    # tile width (atoms)
NQ = NT * M                 # 300 (t, m) chunks
E = NQ * TW                 # 153600 edge slots per core
EPS = 1e-5
BN1_CNT = float(N * M)
BN2_CNT = float(N)

CH = 4                      # gather chunks
CHROWS = 25002              # table rows per chunk
TROWS = NCORES * (NLOC + 1)  # 100008 table rows
ZLOC = 12500                # a zero row local index present in every chunk
NBLK = M + 1                # gather blocks per tile: 12 edge slots + 1 self
NI = TW                     # idxs per dma_gather call (HW limit: 512 works)
NIC = NI // 16              # 32 idx columns per call

AF = mybir.ActivationFunctionType
f32 = mybir.dt.float32
bf16 = mybir.dt.bfloat16
fp8 = mybir.dt.float8e4
i16 = mybir.dt.int16

bf16np = ml_dtypes.bfloat16
fp8np = ml_dtypes.float8_e4m3


def _split_wait_lists(nc, limit=1):
    """This walrus codegen accepts at most one sync wait per instruction on
    several ISA structs; move excess waits onto preceding same-engine NoOp
    carriers (sequential same-engine execution keeps the conjunction)."""
    for bbname, bbw in nc.bb_map.items():
        il = bbw.bb.instructions
        i = 0
        while i < len(il):
            inst = il[i]
            if type(inst).__name__ in ('InstDMAGatherAnt', 'InstLdweights',
                                       'InstMatmult'):
                pass  # these ISA structs take a single wait: split below
            elif inst.engine in (mybir.EngineType.Pool, mybir.EngineType.PE) or \
                    type(inst).__name__ in ('InstISA',):
                i += 1
                continue
            si = getattr(inst, 'sync_info', None)
            waits = list(si.on_wait) if si is not None and si.on_wait is not None else []
            if len(waits) > limit:
                extra, keep = waits[:-limit], waits[-limit:]
                pos = i
                # don't break fused pairs (LdWeights+Matmult): insert before
                # the paired loader
                while pos > 0 and type(il[pos - 1]).__name__ in (
                        'InstLdweights', 'InstTensorLoad'):
                    pos -= 1
                for j in range(0, len(extra), limit):
                    nd = mybir.InstDrain(
                        name=nc.get_next_instruction_name(), ins=[], outs=[])
                    nd.engine = inst.engine
                    nd.sync_info = mybir.SyncInfo(
                        on_wait=list(extra[j:j + limit]), on_update=[])
                    il.insert(pos, nd)
                    pos += 1
                    i += 1
                inst.sync_info = mybir.SyncInfo(
                    on_wait=list(keep), on_update=list(si.on_update))
            i += 1


def _build():
    PH = os.environ.get("KPHASES", "123")
    nc = bass.Bass(num_devices=NCORES, num_swdge_queues=4)

    nbrT = nc.declare_dram_parameter("nbrT", [NT, M, F, TW], fp8, isOutput=False)
    ashard = nc.declare_dram_parameter("ashard", [NLOC + 1, F], bf16, isOutput=False)
    locd = nc.declare_dram_parameter("locd", [NT, 16, NBLK * NIC], i16,
                                     isOutput=False)
    chkd = nc.declare_dram_parameter("chkd", [NT, 16, NBLK * NIC], mybir.dt.uint8,
                                     isOutput=False)
    w_self = nc.declare_dram_parameter("w_self", [F, OUT], bf16, isOutput=False)
    w_g = nc.declare_dram_parameter("w_g", [F, OUT], bf16, isOutput=False)
    w_nb = nc.declare_dram_parameter("w_nb", [F, OUT], bf16, isOutput=False)
    gam1 = nc.declare_dram_parameter("gam1", [OUT, 1], f32, isOutput=False)
    bet1 = nc.declare_dram_parameter("bet1", [OUT, 1], f32, isOutput=False)
    gam2 = nc.declare_dram_parameter("gam2", [F, 1], f32, isOutput=False)
    bet2 = nc.declare_dram_parameter("bet2", [F, 1], f32, isOutput=False)
    delta = nc.declare_dram_parameter("delta", [F, NLOC], bf16, isOutput=True)

    ag_loc = nc.dram_tensor("ag_loc", [NLOC + 1, 128], bf16)
    ag_sh = nc.dram_tensor("ag_sh", [TROWS, 128], bf16, addr_space="Shared")
    y_dram = nc.dram_tensor("y_dram", [128, E], bf16)
    msg_dram = nc.dram_tensor("msg_dram", [F, NPAD], f32)
    st1_loc = nc.dram_tensor("st1_loc", [OUT, 2], f32)
    st1_sh = nc.dram_tensor("st1_sh", [OUT, 2], f32, addr_space="Shared")
    st2_loc = nc.dram_tensor("st2_loc", [F, 2], f32)
    st2_sh = nc.dram_tensor("st2_sh", [F, 2], f32, addr_space="Shared")

    groups = [list(range(NCORES))]

    with TileContext(nc, num_cores=NCORES) as tc:
        with tc.tile_pool(name="const", bufs=1) as cpool, \
             tc.tile_pool(name="work", bufs=4) as pool, \
             tc.tile_pool(name="p23", bufs=2) as pool2, \
             tc.tile_pool(name="idxp", bufs=2) as ipool, \
             tc.tile_pool(name="gat", bufs=6) as gpool, \
             tc.tile_pool(name="acc", bufs=1) as apool, \
             tc.tile_pool(name="psum", bufs=3, space="PSUM") as pp:

            nc.gpsimd.load_library(library_config.mlp)
            nireg = nc.gpsimd.to_reg(NI)

            # --- build the gather table: shard -> AllGather ---
            # spread [12501, 64] shard into rows of the 128-wide (256B) table
            # layout; cols 64:128 stay uninitialized (gather partitions 64:128
            # are never consumed)
            nc.sync.dma_start(out=ag_loc[:, 0:F], in_=ashard[:])
            nc.gpsimd.collective_compute(
                "AllGather", mybir.AluOpType.bypass, replica_groups=groups,
                ins=[ag_loc[:]], outs=[ag_sh[:]])

            # --- constants ---
            ws_sb = cpool.tile([F, OUT], bf16)
            nc.sync.dma_start(out=ws_sb[:], in_=w_self[:])
            wg_sb = cpool.tile([F, OUT], bf16)
            nc.sync.dma_start(out=wg_sb[:], in_=w_g[:])
            wn_sb = cpool.tile([F, OUT], bf16)
            nc.sync.dma_start(out=wn_sb[:], in_=w_nb[:])
            g1_sb = cpool.tile([OUT, 1], f32)
            nc.sync.dma_start(out=g1_sb[:], in_=gam1[:])
            b1_sb = cpool.tile([OUT, 1], f32)
            nc.sync.dma_start(out=b1_sb[:], in_=bet1[:])
            g2_sb = cpool.tile([F, 1], f32)
            nc.sync.dma_start(out=g2_sb[:], in_=gam2[:])
            b2_sb = cpool.tile([F, 1], f32)
            nc.sync.dma_start(out=b2_sb[:], in_=bet2[:])

            ysum = apool.tile([128, NQ], f32)
            ysq = apool.tile([128, NQ], f32)
            msum = apool.tile([F, NT], f32)
            ysq2 = apool.tile([F, NT], f32)

            # --- pass 1 ---
            def gather_block(t, blk, tag_a, tag_b):
                """4 chunk-gathers + tree adds -> [64, TW] bf16 in result[0:64].
                All 4 gathers issue back-to-back on separate SWDGE queues."""
                tiles = []
                for ch in range(CH):
                    T = gpool.tile([128, 1, NI], bf16,
                                   tag=tag_a if ch == 0 else tag_b)
                    nc.gpsimd.dma_gather(
                        T[:], ag_sh[CHROWS * ch:CHROWS * (ch + 1), :],
                        idx_sbs[t][ch][:, blk * NIC:(blk + 1) * NIC],
                        num_idxs=NI, num_idxs_reg=nireg, elem_size=128,
                        transpose=True, queue_num=ch)
                    tiles.append(T)
                A, B1, B2, B3 = tiles
                nc.vector.tensor_add(A[0:64, 0, :], A[0:64, 0, :], B1[0:64, 0, :])
                nc.vector.tensor_add(B2[0:64, 0, :], B2[0:64, 0, :], B3[0:64, 0, :])
                nc.vector.tensor_add(A[0:64, 0, :], A[0:64, 0, :], B2[0:64, 0, :])
                return A

            idx_sbs = {}
            NOGATH = os.environ.get("KNOGATHER", "0") == "1"
            NW = NBLK * NIC
            for t in range(int(os.environ.get("KNT1", NT))):
                if not NOGATH:
                    # expand compressed (local, chunk) -> 4 per-chunk idx tiles:
                    # idx_ch = (chunk == ch) ? local : ZLOC
                    locr = ipool.tile([128, NW], i16, tag="locr")
                    chkr = ipool.tile([128, NW], mybir.dt.uint8, tag="chkr")
                    for g in range(8):
                        nc.sync.dma_start(out=locr[16 * g:16 * (g + 1), :],
                                          in_=locd[t])
                        nc.scalar.dma_start(out=chkr[16 * g:16 * (g + 1), :],
                                            in_=chkd[t])
                    locf = ipool.tile([128, NW], f32, tag="locf")
                    nc.vector.tensor_copy(out=locf[:], in_=locr[:])
                    nc.vector.tensor_scalar_add(locf[:], locf[:], float(-ZLOC))
                    chkf = ipool.tile([128, NW], f32, tag="chkf")
                    nc.vector.tensor_copy(out=chkf[:], in_=chkr[:])
                    idxt = []
                    for ch in range(CH):
                        eq = ipool.tile([128, NW], f32, tag="eq")
                        nc.vector.tensor_scalar(
                            out=eq[:], in0=chkf[:], scalar1=float(ch),
                            scalar2=None, op0=mybir.AluOpType.is_equal)
                        nc.vector.tensor_mul(eq[:], eq[:], locf[:])
                        nc.vector.tensor_scalar_add(eq[:], eq[:], float(ZLOC))
                        ic = ipool.tile([128, NW], i16, tag=f"ic{ch}")
                        nc.vector.tensor_copy(out=ic[:], in_=eq[:])
                        idxt.append(ic)
                    idx_sbs[t] = idxt
                    at = gather_block(t, M, "sA", "sB")
                for m in range(M):
                    q = t * M + m
                    if not NOGATH:
                        Am = gather_block(t, m, "gA", "gB")
                    nb8 = pool.tile([F, TW], fp8, tag="nb8")
                    nc.sync.dma_start(out=nb8[:], in_=nbrT[t, m])
                    nbv = pool.tile([F, TW], bf16, tag="nbv")
                    nc.vector.tensor_copy(out=nbv[:], in_=nb8[:])
                    yp = pp.tile([128, TW], f32, tag="yp")
                    if not NOGATH:
                        nc.tensor.matmul(yp[:], lhsT=ws_sb[:],
                                         rhs=at[0:64, 0, :],
                                         start=True, stop=False)
                        nc.tensor.matmul(yp[:], lhsT=wg_sb[:],
                                         rhs=Am[0:64, 0, :],
                                         start=False, stop=False)
                        nc.tensor.matmul(yp[:], lhsT=wn_sb[:], rhs=nbv[:],
                                         start=False, stop=True)
                    else:
                        nc.tensor.matmul(yp[:], lhsT=wn_sb[:], rhs=nbv[:],
                                         start=True, stop=True)
                    # DVE: psum -> bf16 sbuf copy, fused sum(y) accumulation
                    y_sb = pool.tile([128, TW], bf16, tag="ysb")
                    nc.vector.tensor_scalar(out=y_sb[:], in0=yp[:], scalar1=1.0,
                                            scalar2=0.0, op0=mybir.AluOpType.mult,
                                            op1=mybir.AluOpType.add,
                                            accum_out=ysum[:, q:q + 1])
                    # ACT: sum(y^2) from the bf16 copy
                    sqt = pool.tile([128, TW], f32, tag="sqt")
                    nc.scalar.activation(out=sqt[:], in_=y_sb[:], func=AF.Square,
                                         accum_out=ysq[:, q:q + 1])
                    nc.scalar.dma_start(out=y_dram[:, q * TW:(q + 1) * TW], in_=y_sb[:])

            # --- BN1 stats: reduce + allreduce ---
            st1 = apool.tile([OUT, 2], f32)
            nc.vector.tensor_reduce(st1[:, 0:1], ysum[:], axis=mybir.AxisListType.X,
                                    op=mybir.AluOpType.add)
            nc.vector.tensor_reduce(st1[:, 1:2], ysq[:], axis=mybir.AxisListType.X,
                                    op=mybir.AluOpType.add)
            nc.sync.dma_start(out=st1_loc[:], in_=st1[:])
            nc.gpsimd.collective_compute(
                "AllReduce", mybir.AluOpType.add, replica_groups=groups,
                ins=[st1_loc[:]], outs=[st1_sh[:]])
            st1g = apool.tile([OUT, 2], f32)
            nc.sync.dma_start(out=st1g[:], in_=st1_sh[:])

            # s1 = gam1 / sqrt(var + eps); t1 = bet1 - mean * s1
            mu1 = apool.tile([OUT, 1], f32)
            nc.vector.tensor_scalar_mul(mu1[:], st1g[:, 0:1], 1.0 / BN1_CNT)
            var1 = apool.tile([OUT, 1], f32)
            nc.vector.tensor_scalar_mul(var1[:], st1g[:, 1:2], 1.0 / BN1_CNT)
            musq = apool.tile([OUT, 1], f32)
            nc.vector.tensor_mul(musq[:], mu1[:], mu1[:])
            nc.vector.tensor_sub(var1[:], var1[:], musq[:])
            nc.vector.tensor_scalar_add(var1[:], var1[:], EPS)
            lnv1 = apool.tile([OUT, 1], f32)
            nc.scalar.activation(out=lnv1[:], in_=var1[:], func=AF.Ln)
            nc.vector.tensor_scalar_mul(lnv1[:], lnv1[:], -0.5)
            inv1 = apool.tile([OUT, 1], f32)
            nc.scalar.activation(out=inv1[:], in_=lnv1[:], func=AF.Exp)
            s1 = apool.tile([OUT, 1], f32)
            nc.vector.tensor_mul(s1[:], g1_sb[:], inv1[:])
            t1 = apool.tile([OUT, 1], f32)
            nc.vector.tensor_mul(t1[:], mu1[:], s1[:])
            nc.vector.tensor_sub(t1[:], b1_sb[:], t1[:])

            # stacked scale/shift: [s_f; s_f], [t_f; t_f], [s_c; s_c], [t_c; t_c]
            sf2 = apool.tile([128, 1], f32)
            tf2 = apool.tile([128, 1], f32)
            sc2 = apool.tile([128, 1], f32)
            tc2 = apool.tile([128, 1], f32)
            nc.vector.tensor_copy(out=sf2[0:64, :], in_=s1[0:64, :])
            nc.vector.tensor_copy(out=tf2[0:64, :], in_=t1[0:64, :])
            nc.sync.dma_start(out=sf2[64:128, :], in_=s1[0:64, :])
            nc.sync.dma_start(out=tf2[64:128, :], in_=t1[0:64, :])
            nc.sync.dma_start(out=sc2[0:64, :], in_=s1[64:128, :])
            nc.sync.dma_start(out=tc2[0:64, :], in_=t1[64:128, :])
            nc.sync.dma_start(out=sc2[64:128, :], in_=s1[64:128, :])
            nc.sync.dma_start(out=tc2[64:128, :], in_=t1[64:128, :])

            # --- pass 2: msg = sum_m sigmoid(f)*softplus(c) ---
            for t in range(NT if "2" in PH else 0):
                macc = pool2.tile([128, TW], f32, tag="macc")
                for k in range(M // 2):
                    q0 = (t * M + 2 * k) * TW
                    q1 = (t * M + 2 * k + 1) * TW
                    yf2 = pool2.tile([128, TW], bf16, tag="yf2")
                    nc.sync.dma_start(out=yf2[0:64, :], in_=y_dram[0:64, q0:q0 + TW])
                    nc.sync.dma_start(out=yf2[64:128, :], in_=y_dram[0:64, q1:q1 + TW])
                    yc2 = pool2.tile([128, TW], bf16, tag="yc2")
                    nc.scalar.dma_start(out=yc2[0:64, :], in_=y_dram[64:128, q0:q0 + TW])
                    nc.scalar.dma_start(out=yc2[64:128, :], in_=y_dram[64:128, q1:q1 + TW])
                    sg = pool2.tile([128, TW], f32, tag="sg")
                    nc.scalar.activation(out=sg[:], in_=yf2[:], func=AF.Sigmoid,
                                         bias=tf2[:, 0:1], scale=sf2[:, 0:1])
                    ec = pool2.tile([128, TW], f32, tag="ec")
                    nc.scalar.activation(out=ec[:], in_=yc2[:], func=AF.Exp,
                                         bias=tc2[:, 0:1], scale=sc2[:, 0:1])
                    sp = pool2.tile([128, TW], f32, tag="sp")
                    nc.scalar.activation(out=sp[:], in_=ec[:], func=AF.Ln, bias=1.0)
                    if k == 0:
                        nc.vector.tensor_mul(macc[:], sg[:], sp[:])
                    else:
                        prod = pool2.tile([128, TW], f32, tag="prod")
                        nc.vector.tensor_mul(prod[:], sg[:], sp[:])
                        nc.vector.tensor_add(macc[:], macc[:], prod[:])
                # fold top (m even) + bottom (m odd): shift bottom to partitions 0-63
                mlo = pool2.tile([64, TW], f32, tag="mlo")
                nc.sync.dma_start(out=mlo[:], in_=macc[64:128, :])
                mout = pool2.tile([64, TW], f32, tag="mout")
                nc.vector.tensor_add(mout[:], macc[0:64, :], mlo[:])
                if t == NT - 1:
                    nc.vector.memset(mout[:, NLOC - t * TW:TW], 0.0)
                # per-tile BN2 stat accumulation + spill msg tile to DRAM
                nc.vector.tensor_reduce(msum[:, t:t + 1], mout[:],
                                        axis=mybir.AxisListType.X,
                                        op=mybir.AluOpType.add)
                sq2t = pool2.tile([F, TW], f32, tag="sq2t")
                nc.scalar.activation(out=sq2t[:], in_=mout[:], func=AF.Square,
                                     accum_out=ysq2[:, t:t + 1])
                nc.sync.dma_start(out=msg_dram[:, t * TW:(t + 1) * TW], in_=mout[:])

            if "2" not in PH:
                nc.vector.memset(msum[:], 1.0)
                nc.vector.memset(ysq2[:], 1.0)
            st2 = apool.tile([F, 2], f32)
            nc.vector.tensor_reduce(st2[:, 0:1], msum[:], axis=mybir.AxisListType.X,
                                    op=mybir.AluOpType.add)
            nc.vector.tensor_reduce(st2[:, 1:2], ysq2[:], axis=mybir.AxisListType.X,
                                    op=mybir.AluOpType.add)
            nc.sync.dma_start(out=st2_loc[:], in_=st2[:])
            nc.gpsimd.collective_compute(
                "AllReduce", mybir.AluOpType.add, replica_groups=groups,
                ins=[st2_loc[:]], outs=[st2_sh[:]])
            st2g = apool.tile([F, 2], f32)
            nc.sync.dma_start(out=st2g[:], in_=st2_sh[:])

            mu2 = apool.tile([F, 1], f32)
            nc.vector.tensor_scalar_mul(mu2[:], st2g[:, 0:1], 1.0 / BN2_CNT)
            var2 = apool.tile([F, 1], f32)
            nc.vector.tensor_scalar_mul(var2[:], st2g[:, 1:2], 1.0 / BN2_CNT)
            msq2 = apool.tile([F, 1], f32)
            nc.vector.tensor_mul(msq2[:], mu2[:], mu2[:])
            nc.vector.tensor_sub(var2[:], var2[:], msq2[:])
            nc.vector.tensor_scalar_add(var2[:], var2[:], EPS)
            lnv2 = apool.tile([F, 1], f32)
            nc.scalar.activation(out=lnv2[:], in_=var2[:], func=AF.Ln)
            nc.vector.tensor_scalar_mul(lnv2[:], lnv2[:], -0.5)
            inv2 = apool.tile([F, 1], f32)
            nc.scalar.activation(out=inv2[:], in_=lnv2[:], func=AF.Exp)
            s2 = apool.tile([F, 1], f32)
            nc.vector.tensor_mul(s2[:], g2_sb[:], inv2[:])
            t2 = apool.tile([F, 1], f32)
            nc.vector.tensor_mul(t2[:], mu2[:], s2[:])
            nc.vector.tensor_sub(t2[:], b2_sb[:], t2[:])

            # --- pass 3: delta = softplus(bn2(msg)) -> bf16 ---
            for t in range(NT if "3" in PH else 1):
                mtin = pool2.tile([64, TW], f32, tag="mtin")
                nc.sync.dma_start(out=mtin[:], in_=msg_dram[:, t * TW:(t + 1) * TW])
                ex3 = pool2.tile([64, TW], f32, tag="ex3")
                nc.scalar.activation(out=ex3[:], in_=mtin[:], func=AF.Exp,
                                     bias=t2[:, 0:1], scale=s2[:, 0:1])
                d_sb = pool2.tile([64, TW], bf16, tag="dsb")
                nc.scalar.activation(out=d_sb[:], in_=ex3[:], func=AF.Ln, bias=1.0)
                w = min(TW, NLOC - t * TW)
                nc.sync.dma_start(out=delta[:, t * TW:t * TW + w], in_=d_sb[:, 0:w])

    _split_wait_lists(nc)
    mybir.codegen_inst_isa_subclasses(nc)
    return nc


_NC_CACHE = None


def _get_nc():
    global _NC_CACHE
    if _NC_CACHE is None:
        _NC_CACHE = _build()
    return _NC_CACHE


def _prep_in_maps(atom_fea, nbr_fea, nbr_idx, W_full,
                  bn1_gamma, bn1_beta, bn2_gamma, bn2_beta):
    atom_fea = np.asarray(atom_fea, np.float32)
    nbr_fea = np.asarray(nbr_fea, np.float32)
    nbr_idx = np.asarray(nbr_idx).astype(np.int64)
    W_full = np.asarray(W_full, np.float32)

    # global edge chunk/local indices: table row of atom j is r = j + j//12500
    r_all = (nbr_idx + nbr_idx // NLOC).astype(np.int32)       # [N, M]
    c_all = r_all // CHROWS
    l_all = (r_all - c_all * CHROWS).astype(np.int16)

    shared = {
        "w_self": np.ascontiguousarray(W_full[0:64]).astype(bf16np),
        "w_g": np.ascontiguousarray(W_full[64:128]).astype(bf16np),
        "w_nb": np.ascontiguousarray(W_full[128:192]).astype(bf16np),
        "gam1": np.asarray(bn1_gamma, np.float32).reshape(OUT, 1).copy(),
        "bet1": np.asarray(bn1_beta, np.float32).reshape(OUT, 1).copy(),
        "gam2": np.asarray(bn2_gamma, np.float32).reshape(F, 1).copy(),
        "bet2": np.asarray(bn2_beta, np.float32).reshape(F, 1).copy(),
    }

    in_maps = []
    for c in range(NCORES):
        lo = c * NLOC
        # ashard: [12501, 64] bf16, last row zero (device spreads to 128-wide)
        ash = np.zeros((NLOC + 1, F), bf16np)
        ash[:NLOC] = atom_fea[lo:lo + NLOC]

        # nbrT: [NT, M, F, TW] fp8, pad atoms zero
        nfs = np.zeros((NPAD, M, F), np.float32)
        nfs[:NLOC] = nbr_fea[lo:lo + NLOC]
        nbrT = np.ascontiguousarray(
            nfs.reshape(NT, TW, M, F).transpose(0, 2, 3, 1)).astype(fp8np)

        # compressed idx: per edge a chunk id (255 = none -> zero row in every
        # chunk) and a chunk-local row; device expands to 4 per-chunk arrays
        ce = np.full((NPAD, M), 255, np.uint8)
        le = np.full((NPAD, M), ZLOC, np.int16)
        ce[:NLOC] = c_all[lo:lo + NLOC]
        le[:NLOC] = l_all[lo:lo + NLOC]

        # self slots: local atom a -> chunk c//2, local a + 12501*(c%2);
        # pad atoms -> that chunk's zero row
        a = np.arange(NPAD, dtype=np.int32)
        sl = np.where(a < NLOC, a + (NLOC + 1) * (c % 2),
                      ZLOC + (NLOC + 1) * (c % 2)).astype(np.int16)
        sc = np.full(NPAD, c // 2, np.uint8)

        # [NT, NBLK, TW] in block order (m blocks then self)
        L = np.concatenate([le.reshape(NT, TW, M).transpose(0, 2, 1),
                            sl.reshape(NT, 1, TW)], axis=1)
        C = np.concatenate([ce.reshape(NT, TW, M).transpose(0, 2, 1),
                            sc.reshape(NT, 1, TW)], axis=1)
        # wrapped: idx position i -> partition i%16, column i//16
        locd = np.ascontiguousarray(
            L.reshape(NT, NBLK, NIC, 16).transpose(0, 3, 1, 2)
            .reshape(NT, 16, NBLK * NIC))
        chkd = np.ascontiguousarray(
            C.reshape(NT, NBLK, NIC, 16).transpose(0, 3, 1, 2)
            .reshape(NT, 16, NBLK * NIC))

        m = {"ashard": ash, "nbrT": nbrT, "locd": locd, "chkd": chkd}
        m.update(shared)
        in_maps.append(m)
    return in_maps


def kernel(atom_fea, nbr_fea, nbr_idx, W_full, b_full,
           bn1_gamma, bn1_beta, bn2_gamma, bn2_beta):
    atom_fea = np.asarray(atom_fea, np.float32)
    in_maps = _prep_in_maps(atom_fea, nbr_fea, nbr_idx, W_full,
                            bn1_gamma, bn1_beta, bn2_gamma, bn2_beta)
    nc = _get_nc()
    res = run_bass_kernel_spmd(nc, in_maps, list(range(NCORES)))
    out = np.empty((N, F), np.float32)
    for c in range(NCORES):
        d = res.results[c]["delta"].astype(np.float32)          # [64, NLOC]
        out[c * NLOC:(c + 1) * NLOC] = d.T
    out += atom_fea
    return out


# revision 49
# speedup vs baseline: 1.2040x; 1.2040x over previous
"""CGCNNConv on 8 TRN2 NeuronCores — transfer-optimized.

The axon tunnel (~63 MB/s effective) dominates wall time, so the design
minimizes host<->device bytes:
  - nbr_fea ships as fp8(e4m3), feature-major [NT, M, 64, TW] (9.8MB/core).
  - atom features ship SHARDED: each core sends its [12501, 128] bf16
    zero-padded slice (cols 64:128 zero, last row zero); a device AllGather
    builds the full [100008, 128] bf16 table once.  Row of atom j is
    r = j + j//12500 (one zero row after each shard); chunk c = r//25002
    gives 4 int16-addressable gather chunks, each with zero rows at local
    12500 and 25001 (used as the "no contribution" target).
  - neighbor gathers run ON DEVICE via gpsimd dma_gather (transpose mode,
    256B rows -> features land partition-major), 4 chunk-gathers per tile
    (6656 idxs = 12*512 edge slots + 512 self slots), summed with 3 DVE adds.
  - output is just delta = softplus(bn2(msg)) as [64, NPAD] bf16; the final
    atom_fea + delta add happens on host in f32.
Device per core, single NEFF:
  pass1: per t: 4 dma_gathers + adds -> gathered/self features; per (t,m):
         3 K=64 matmuls (self, gathered, nbr) -> PSUM [128, 512];
         DVE copies y->bf16 (+sum(y)), ACT accumulates sum(y^2),
         y stored to DRAM scratch [128, 153600] bf16.
  AllReduce [128, 2] -> BN1 scale/shift (linear bias b folds out).
  pass2: reload y halves stacked, ACT sigmoid/softplus, DVE -> msg [64, NPAD].
  AllReduce [64, 2] -> BN2; pass3: softplus(bn2(msg)) -> bf16 delta out.
"""
import os
import sys
sys.path.insert(0, '/opt/trn_rl_repo')
import numpy as np
import ml_dtypes

from concourse import bass, mybir
from concourse import library_config
from concourse.tile import TileContext
from concourse.bass_utils import run_bass_kernel_spmd

NCORES = 8
N = 100000
M = 12
F = 64                      # atom/nbr feature len
OUT = 128                   # 2F
NLOC = 12500                # atoms per core
NPAD = 12800                # padded atoms per core
NT = 25                     # tiles of 512 atoms
TW = 512                    # tile width (atoms)
NQ = NT * M                 # 300 (t, m) chunks
E = NQ * TW                 # 153600 edge slots per core
EPS = 1e-5
BN1_CNT = float(N * M)
BN2_CNT = float(N)

CH = 4                      # gather chunks
CHROWS = 25002              # table rows per chunk
TROWS = NCORES * (NLOC + 1)  # 100008 table rows
NBLK = M + 1                # gather blocks per tile: 12 edge slots + 1 self
NI = TW                     # idxs per dma_gather call (HW limit: 512 works)
NIC = NI // 16              # 32 idx columns per call
NIE = M * NIC               # idx columns of the edge blocks

AF = mybir.ActivationFunctionType
f32 = mybir.dt.float32
bf16 = mybir.dt.bfloat16
fp8 = mybir.dt.float8e4
i16 = mybir.dt.int16

bf16np = ml_dtypes.bfloat16
fp8np = ml_dtypes.float8_e4m3


def _split_wait_lists(nc, limit=1):
    """This walrus codegen accepts at most one sync wait per instruction on
    several ISA structs; move excess waits onto preceding same-engine NoOp
    carriers (sequential same-engine execution keeps the conjunction)."""
    for bbname, bbw in nc.bb_map.items():
        il = bbw.bb.instructions
        i = 0
        while i < len(il):
            inst = il[i]
            if type(inst).__name__ in ('InstDMAGatherAnt', 'InstLdweights',
                                       'InstMatmult'):
                pass  # these ISA structs take a single wait: split below
            elif inst.engine in (mybir.EngineType.Pool, mybir.EngineType.PE) or \
                    type(inst).__name__ in ('InstISA',):
                i += 1
                continue
            si = getattr(inst, 'sync_info', None)
            waits = list(si.on_wait) if si is not None and si.on_wait is not None else []
            if len(waits) > limit:
                extra, keep = waits[:-limit], waits[-limit:]
                pos = i
                # don't break fused pairs (LdWeights+Matmult): insert before
                # the paired loader
                while pos > 0 and type(il[pos - 1]).__name__ in (
                        'InstLdweights', 'InstTensorLoad'):
                    pos -= 1
                for j in range(0, len(extra), limit):
                    nd = mybir.InstDrain(
                        name=nc.get_next_instruction_name(), ins=[], outs=[])
                    nd.engine = inst.engine
                    nd.sync_info = mybir.SyncInfo(
                        on_wait=list(extra[j:j + limit]), on_update=[])
                    il.insert(pos, nd)
                    pos += 1
                    i += 1
                inst.sync_info = mybir.SyncInfo(
                    on_wait=list(keep), on_update=list(si.on_update))
            i += 1


def _build():
    PH = os.environ.get("KPHASES", "123")
    nc = bass.Bass(num_devices=NCORES, num_swdge_queues=4)

    nbrT = nc.declare_dram_parameter("nbrT", [NT, M, F, TW], fp8, isOutput=False)
    ashard = nc.declare_dram_parameter("ashard", [NLOC + 1, F], bf16, isOutput=False)
    locd = nc.declare_dram_parameter("locd", [NT, 16, NBLK * NIC], i16,
                                     isOutput=False)
    chkd = nc.declare_dram_parameter("chkd", [NT, 16, NBLK * NIC], mybir.dt.uint8,
                                     isOutput=False)

    w_self = nc.declare_dram_parameter("w_self", [F, OUT], bf16, isOutput=False)
    w_g = nc.declare_dram_parameter("w_g", [F, OUT], bf16, isOutput=False)
    w_nb = nc.declare_dram_parameter("w_nb", [F, OUT], bf16, isOutput=False)
    gam1 = nc.declare_dram_parameter("gam1", [OUT, 1], f32, isOutput=False)
    bet1 = nc.declare_dram_parameter("bet1", [OUT, 1], f32, isOutput=False)
    gam2 = nc.declare_dram_parameter("gam2", [F, 1], f32, isOutput=False)
    bet2 = nc.declare_dram_parameter("bet2", [F, 1], f32, isOutput=False)
    delta = nc.declare_dram_parameter("delta", [F, NLOC], bf16, isOutput=True)

    ag_loc = nc.dram_tensor("ag_loc", [NLOC + 1, 128], bf16)
    ag_sh = nc.dram_tensor("ag_sh", [TROWS, 128], bf16, addr_space="Shared")
    y_dram = nc.dram_tensor("y_dram", [128, E], bf16)
    msg_dram = nc.dram_tensor("msg_dram", [F, NPAD], f32)
    st1_loc = nc.dram_tensor("st1_loc", [OUT, 2], f32)
    st1_sh = nc.dram_tensor("st1_sh", [OUT, 2], f32, addr_space="Shared")
    st2_loc = nc.dram_tensor("st2_loc", [F, 2], f32)
    st2_sh = nc.dram_tensor("st2_sh", [F, 2], f32, addr_space="Shared")

    groups = [list(range(NCORES))]

    with TileContext(nc, num_cores=NCORES) as tc:
        with tc.tile_pool(name="const", bufs=1) as cpool, \
             tc.tile_pool(name="work", bufs=4) as pool, \
             tc.tile_pool(name="p23", bufs=2) as pool2, \
             tc.tile_pool(name="idxp", bufs=2) as ipool, \
             tc.tile_pool(name="gat", bufs=6) as gpool, \
             tc.tile_pool(name="acc", bufs=1) as apool, \
             tc.tile_pool(name="psum", bufs=3, space="PSUM") as pp:

            nc.gpsimd.load_library(library_config.mlp)
            nireg = nc.gpsimd.to_reg(NI)

            # --- build the gather table: shard -> AllGather ---
            # spread [12501, 64] shard into rows of the 128-wide (256B) table
            # layout; cols 64:128 stay uninitialized (gather partitions 64:128
            # are never consumed)
            nc.sync.dma_start(out=ag_loc[:, 0:F], in_=ashard[:])
            nc.gpsimd.collective_compute(
                "AllGather", mybir.AluOpType.bypass, replica_groups=groups,
                ins=[ag_loc[:]], outs=[ag_sh[:]])

            # --- constants ---
            ws_sb = cpool.tile([F, OUT], bf16)
            nc.sync.dma_start(out=ws_sb[:], in_=w_self[:])
            wg_sb = cpool.tile([F, OUT], bf16)
            nc.sync.dma_start(out=wg_sb[:], in_=w_g[:])
            wn_sb = cpool.tile([F, OUT], bf16)
            nc.sync.dma_start(out=wn_sb[:], in_=w_nb[:])
            g1_sb = cpool.tile([OUT, 1], f32)
            nc.sync.dma_start(out=g1_sb[:], in_=gam1[:])
            b1_sb = cpool.tile([OUT, 1], f32)
            nc.sync.dma_start(out=b1_sb[:], in_=bet1[:])
            g2_sb = cpool.tile([F, 1], f32)
            nc.sync.dma_start(out=g2_sb[:], in_=gam2[:])
            b2_sb = cpool.tile([F, 1], f32)
            nc.sync.dma_start(out=b2_sb[:], in_=bet2[:])

            ysum = apool.tile([128, NQ], f32)
            ysq = apool.tile([128, NQ], f32)
            msum = apool.tile([F, NT], f32)
            ysq2 = apool.tile([F, NT], f32)

            # --- pass 1 ---
            def gather_block(t, blk, tag_a, tag_b):
                """4 chunk-gathers + tree adds -> [64, TW] bf16 in result[0:64].
                All 4 gathers issue back-to-back on separate SWDGE queues.
                Out-of-chunk / pad slots use idx 0 = the chunk-base zero row."""
                tiles = []
                for ch in range(CH):
                    T = gpool.tile([128, 1, NI], bf16,
                                   tag=tag_a if ch == 0 else tag_b)
                    nc.gpsimd.dma_gather(
                        T[:], ag_sh[CHROWS * ch:CHROWS * (ch + 1), :],
                        idx_sbs[t][ch][:, blk * NIC:(blk + 1) * NIC],
                        num_idxs=NI, num_idxs_reg=nireg, elem_size=128,
                        transpose=True, queue_num=ch)
                    tiles.append(T)
                A, B1, B2, B3 = tiles
                nc.vector.tensor_add(A[0:64, 0, :], A[0:64, 0, :], B1[0:64, 0, :])
                nc.vector.tensor_add(B2[0:64, 0, :], B2[0:64, 0, :], B3[0:64, 0, :])
                nc.vector.tensor_add(A[0:64, 0, :], A[0:64, 0, :], B2[0:64, 0, :])
                return A

            idx_sbs = {}
            NOGATH = os.environ.get("KNOGATHER", "0") == "1"
            NW = NBLK * NIC
            for t in range(int(os.environ.get("KNT1", NT))):
                if not NOGATH:
                    # expand compressed (local, chunk) -> 4 per-chunk idx
                    # tiles: idx = (chunk == ch) * local -> local in its
                    # chunk, 0 (= the chunk-base zero row) elsewhere.
                    locr = ipool.tile([128, NW], i16, tag="locr")
                    chkr = ipool.tile([128, NW], mybir.dt.uint8, tag="chkr")
                    for g in range(8):
                        nc.sync.dma_start(out=locr[16 * g:16 * (g + 1), :],
                                          in_=locd[t])
                        nc.scalar.dma_start(out=chkr[16 * g:16 * (g + 1), :],
                                            in_=chkd[t])
                    locf = ipool.tile([128, NW], f32, tag="locf")
                    nc.vector.tensor_copy(out=locf[:], in_=locr[:])
                    chkf = ipool.tile([128, NW], f32, tag="chkf")
                    nc.vector.tensor_copy(out=chkf[:], in_=chkr[:])
                    idxt = []
                    for ch in range(CH):
                        eq = ipool.tile([128, NW], f32, tag="eq")
                        nc.vector.tensor_scalar(
                            out=eq[:], in0=chkf[:], scalar1=float(ch),
                            scalar2=None, op0=mybir.AluOpType.is_equal)
                        nc.vector.tensor_mul(eq[:], eq[:], locf[:])
                        ic = ipool.tile([128, NW], i16, tag=f"ic{ch}")
                        nc.vector.tensor_copy(out=ic[:], in_=eq[:])
                        idxt.append(ic)
                    idx_sbs[t] = idxt
                    at = gather_block(t, M, "sA", "sB")
                for m in range(M):
                    q = t * M + m
                    if not NOGATH:
                        Am = gather_block(t, m, "gA", "gB")
                    nb8 = pool.tile([F, TW], fp8, tag="nb8")
                    nc.sync.dma_start(out=nb8[:], in_=nbrT[t, m])
                    nbv = pool.tile([F, TW], bf16, tag="nbv")
                    nc.vector.tensor_copy(out=nbv[:], in_=nb8[:])
                    yp = pp.tile([128, TW], f32, tag="yp")
                    if not NOGATH:
                        nc.tensor.matmul(yp[:], lhsT=ws_sb[:],
                                         rhs=at[0:64, 0, :],
                                         start=True, stop=False)
                        nc.tensor.matmul(yp[:], lhsT=wg_sb[:],
                                         rhs=Am[0:64, 0, :],
                                         start=False, stop=False)
                        nc.tensor.matmul(yp[:], lhsT=wn_sb[:], rhs=nbv[:],
                                         start=False, stop=True)
                    else:
                        nc.tensor.matmul(yp[:], lhsT=wn_sb[:], rhs=nbv[:],
                                         start=True, stop=True)
                    # DVE: psum -> bf16 sbuf copy, fused sum(y) accumulation
                    y_sb = pool.tile([128, TW], bf16, tag="ysb")
                    nc.vector.tensor_scalar(out=y_sb[:], in0=yp[:], scalar1=1.0,
                                            scalar2=0.0, op0=mybir.AluOpType.mult,
                                            op1=mybir.AluOpType.add,
                                            accum_out=ysum[:, q:q + 1])
                    # ACT: sum(y^2) from the bf16 copy
                    sqt = pool.tile([128, TW], f32, tag="sqt")
                    nc.scalar.activation(out=sqt[:], in_=y_sb[:], func=AF.Square,
                                         accum_out=ysq[:, q:q + 1])
                    nc.scalar.dma_start(out=y_dram[:, q * TW:(q + 1) * TW], in_=y_sb[:])

            # --- BN1 stats: reduce + allreduce ---
            st1 = apool.tile([OUT, 2], f32)
            nc.vector.tensor_reduce(st1[:, 0:1], ysum[:], axis=mybir.AxisListType.X,
                                    op=mybir.AluOpType.add)
            nc.vector.tensor_reduce(st1[:, 1:2], ysq[:], axis=mybir.AxisListType.X,
                                    op=mybir.AluOpType.add)
            nc.sync.dma_start(out=st1_loc[:], in_=st1[:])
            nc.gpsimd.collective_compute(
                "AllReduce", mybir.AluOpType.add, replica_groups=groups,
                ins=[st1_loc[:]], outs=[st1_sh[:]])
            st1g = apool.tile([OUT, 2], f32)
            nc.sync.dma_start(out=st1g[:], in_=st1_sh[:])

            # s1 = gam1 / sqrt(var + eps); t1 = bet1 - mean * s1
            mu1 = apool.tile([OUT, 1], f32)
            nc.vector.tensor_scalar_mul(mu1[:], st1g[:, 0:1], 1.0 / BN1_CNT)
            var1 = apool.tile([OUT, 1], f32)
            nc.vector.tensor_scalar_mul(var1[:], st1g[:, 1:2], 1.0 / BN1_CNT)
            musq = apool.tile([OUT, 1], f32)
            nc.vector.tensor_mul(musq[:], mu1[:], mu1[:])
            nc.vector.tensor_sub(var1[:], var1[:], musq[:])
            nc.vector.tensor_scalar_add(var1[:], var1[:], EPS)
            lnv1 = apool.tile([OUT, 1], f32)
            nc.scalar.activation(out=lnv1[:], in_=var1[:], func=AF.Ln)
            nc.vector.tensor_scalar_mul(lnv1[:], lnv1[:], -0.5)
            inv1 = apool.tile([OUT, 1], f32)
            nc.scalar.activation(out=inv1[:], in_=lnv1[:], func=AF.Exp)
            s1 = apool.tile([OUT, 1], f32)
            nc.vector.tensor_mul(s1[:], g1_sb[:], inv1[:])
            t1 = apool.tile([OUT, 1], f32)
            nc.vector.tensor_mul(t1[:], mu1[:], s1[:])
            nc.vector.tensor_sub(t1[:], b1_sb[:], t1[:])

            # stacked scale/shift: [s_f; s_f], [t_f; t_f], [s_c; s_c], [t_c; t_c]
            sf2 = apool.tile([128, 1], f32)
            tf2 = apool.tile([128, 1], f32)
            sc2 = apool.tile([128, 1], f32)
            tc2 = apool.tile([128, 1], f32)
            nc.vector.tensor_copy(out=sf2[0:64, :], in_=s1[0:64, :])
            nc.vector.tensor_copy(out=tf2[0:64, :], in_=t1[0:64, :])
            nc.sync.dma_start(out=sf2[64:128, :], in_=s1[0:64, :])
            nc.sync.dma_start(out=tf2[64:128, :], in_=t1[0:64, :])
            nc.sync.dma_start(out=sc2[0:64, :], in_=s1[64:128, :])
            nc.sync.dma_start(out=tc2[0:64, :], in_=t1[64:128, :])
            nc.sync.dma_start(out=sc2[64:128, :], in_=s1[64:128, :])
            nc.sync.dma_start(out=tc2[64:128, :], in_=t1[64:128, :])

            # --- pass 2: msg = sum_m sigmoid(f)*softplus(c) ---
            for t in range(NT if "2" in PH else 0):
                macc = pool2.tile([128, TW], f32, tag="macc")
                for k in range(M // 2):
                    q0 = (t * M + 2 * k) * TW
                    q1 = (t * M + 2 * k + 1) * TW
                    yf2 = pool2.tile([128, TW], bf16, tag="yf2")
                    nc.sync.dma_start(out=yf2[0:64, :], in_=y_dram[0:64, q0:q0 + TW])
                    nc.sync.dma_start(out=yf2[64:128, :], in_=y_dram[0:64, q1:q1 + TW])
                    yc2 = pool2.tile([128, TW], bf16, tag="yc2")
                    nc.scalar.dma_start(out=yc2[0:64, :], in_=y_dram[64:128, q0:q0 + TW])
                    nc.scalar.dma_start(out=yc2[64:128, :], in_=y_dram[64:128, q1:q1 + TW])
                    sg = pool2.tile([128, TW], f32, tag="sg")
                    nc.scalar.activation(out=sg[:], in_=yf2[:], func=AF.Sigmoid,
                                         bias=tf2[:, 0:1], scale=sf2[:, 0:1])
                    ec = pool2.tile([128, TW], f32, tag="ec")
                    nc.scalar.activation(out=ec[:], in_=yc2[:], func=AF.Exp,
                                         bias=tc2[:, 0:1], scale=sc2[:, 0:1])
                    sp = pool2.tile([128, TW], f32, tag="sp")
                    nc.scalar.activation(out=sp[:], in_=ec[:], func=AF.Ln, bias=1.0)
                    if k == 0:
                        nc.vector.tensor_mul(macc[:], sg[:], sp[:])
                    else:
                        prod = pool2.tile([128, TW], f32, tag="prod")
                        nc.vector.tensor_mul(prod[:], sg[:], sp[:])
                        nc.vector.tensor_add(macc[:], macc[:], prod[:])
                # fold top (m even) + bottom (m odd): shift bottom to partitions 0-63
                mlo = pool2.tile([64, TW], f32, tag="mlo")
                nc.sync.dma_start(out=mlo[:], in_=macc[64:128, :])
                mout = pool2.tile([64, TW], f32, tag="mout")
                nc.vector.tensor_add(mout[:], macc[0:64, :], mlo[:])
                if t == NT - 1:
                    nc.vector.memset(mout[:, NLOC - t * TW:TW], 0.0)
                # per-tile BN2 stat accumulation + spill msg tile to DRAM
                nc.vector.tensor_reduce(msum[:, t:t + 1], mout[:],
                                        axis=mybir.AxisListType.X,
                                        op=mybir.AluOpType.add)
                sq2t = pool2.tile([F, TW], f32, tag="sq2t")
                nc.scalar.activation(out=sq2t[:], in_=mout[:], func=AF.Square,
                                     accum_out=ysq2[:, t:t + 1])
                nc.sync.dma_start(out=msg_dram[:, t * TW:(t + 1) * TW], in_=mout[:])

            if "2" not in PH:
                nc.vector.memset(msum[:], 1.0)
                nc.vector.memset(ysq2[:], 1.0)
            st2 = apool.tile([F, 2], f32)
            nc.vector.tensor_reduce(st2[:, 0:1], msum[:], axis=mybir.AxisListType.X,
                                    op=mybir.AluOpType.add)
            nc.vector.tensor_reduce(st2[:, 1:2], ysq2[:], axis=mybir.AxisListType.X,
                                    op=mybir.AluOpType.add)
            nc.sync.dma_start(out=st2_loc[:], in_=st2[:])
            nc.gpsimd.collective_compute(
                "AllReduce", mybir.AluOpType.add, replica_groups=groups,
                ins=[st2_loc[:]], outs=[st2_sh[:]])
            st2g = apool.tile([F, 2], f32)
            nc.sync.dma_start(out=st2g[:], in_=st2_sh[:])

            mu2 = apool.tile([F, 1], f32)
            nc.vector.tensor_scalar_mul(mu2[:], st2g[:, 0:1], 1.0 / BN2_CNT)
            var2 = apool.tile([F, 1], f32)
            nc.vector.tensor_scalar_mul(var2[:], st2g[:, 1:2], 1.0 / BN2_CNT)
            msq2 = apool.tile([F, 1], f32)
            nc.vector.tensor_mul(msq2[:], mu2[:], mu2[:])
            nc.vector.tensor_sub(var2[:], var2[:], msq2[:])
            nc.vector.tensor_scalar_add(var2[:], var2[:], EPS)
            lnv2 = apool.tile([F, 1], f32)
            nc.scalar.activation(out=lnv2[:], in_=var2[:], func=AF.Ln)
            nc.vector.tensor_scalar_mul(lnv2[:], lnv2[:], -0.5)
            inv2 = apool.tile([F, 1], f32)
            nc.scalar.activation(out=inv2[:], in_=lnv2[:], func=AF.Exp)
            s2 = apool.tile([F, 1], f32)
            nc.vector.tensor_mul(s2[:], g2_sb[:], inv2[:])
            t2 = apool.tile([F, 1], f32)
            nc.vector.tensor_mul(t2[:], mu2[:], s2[:])
            nc.vector.tensor_sub(t2[:], b2_sb[:], t2[:])

            # --- pass 3: delta = softplus(bn2(msg)) -> bf16 ---
            for t in range(NT if "3" in PH else 1):
                mtin = pool2.tile([64, TW], f32, tag="mtin")
                nc.sync.dma_start(out=mtin[:], in_=msg_dram[:, t * TW:(t + 1) * TW])
                ex3 = pool2.tile([64, TW], f32, tag="ex3")
                nc.scalar.activation(out=ex3[:], in_=mtin[:], func=AF.Exp,
                                     bias=t2[:, 0:1], scale=s2[:, 0:1])
                d_sb = pool2.tile([64, TW], bf16, tag="dsb")
                nc.scalar.activation(out=d_sb[:], in_=ex3[:], func=AF.Ln, bias=1.0)
                w = min(TW, NLOC - t * TW)
                nc.sync.dma_start(out=delta[:, t * TW:t * TW + w], in_=d_sb[:, 0:w])

    _split_wait_lists(nc)
    mybir.codegen_inst_isa_subclasses(nc)
    return nc


_NC_CACHE = None


def _get_nc():
    global _NC_CACHE
    if _NC_CACHE is None:
        _NC_CACHE = _build()
    return _NC_CACHE


def _prep_in_maps(atom_fea, nbr_fea, nbr_idx, W_full,
                  bn1_gamma, bn1_beta, bn2_gamma, bn2_beta):
    atom_fea = np.asarray(atom_fea, np.float32)
    nbr_fea = np.asarray(nbr_fea, np.float32)
    nbr_idx = np.asarray(nbr_idx).astype(np.int64)
    W_full = np.asarray(W_full, np.float32)

    # global edge chunk/local indices: table row of atom j is
    # r = j + j//12500 + 1 (a zero row heads each shard; chunk bases are zero)
    r_all = (nbr_idx + nbr_idx // NLOC + 1).astype(np.int32)   # [N, M]
    c_all = r_all // CHROWS
    l_all = (r_all - c_all * CHROWS).astype(np.int16)

    shared = {
        "w_self": np.ascontiguousarray(W_full[0:64]).astype(bf16np),
        "w_g": np.ascontiguousarray(W_full[64:128]).astype(bf16np),
        "w_nb": np.ascontiguousarray(W_full[128:192]).astype(bf16np),
        "gam1": np.asarray(bn1_gamma, np.float32).reshape(OUT, 1).copy(),
        "bet1": np.asarray(bn1_beta, np.float32).reshape(OUT, 1).copy(),
        "gam2": np.asarray(bn2_gamma, np.float32).reshape(F, 1).copy(),
        "bet2": np.asarray(bn2_beta, np.float32).reshape(F, 1).copy(),
    }

    in_maps = []
    for c in range(NCORES):
        lo = c * NLOC
        # ashard: [12501, 64] bf16, ZERO row first (device spreads to 128-wide)
        ash = np.zeros((NLOC + 1, F), bf16np)
        ash[1:] = atom_fea[lo:lo + NLOC]

        # nbrT: [NT, M, F, TW] fp8, pad atoms zero
        nfs = np.zeros((NPAD, M, F), np.float32)
        nfs[:NLOC] = nbr_fea[lo:lo + NLOC]
        nbrT = np.ascontiguousarray(
            nfs.reshape(NT, TW, M, F).transpose(0, 2, 3, 1)).astype(fp8np)

        # compressed idx: per edge a chunk id (255 = none -> idx 0, the
        # chunk-base zero row) and a chunk-local row; device expands to
        # idx = (chunk == ch) * local per chunk
        ce = np.full((NPAD, M), 255, np.uint8)
        le = np.zeros((NPAD, M), np.int16)
        ce[:NLOC] = c_all[lo:lo + NLOC]
        le[:NLOC] = l_all[lo:lo + NLOC]

        # self slots: local atom a -> chunk c//2, local a+1 + 12501*(c%2);
        # pad atoms -> 0 (chunk-base zero row)
        a = np.arange(NPAD, dtype=np.int32)
        sl = np.where(a < NLOC, a + 1 + (NLOC + 1) * (c % 2), 0).astype(np.int16)
        sc = np.full(NPAD, c // 2, np.uint8)

        # [NT, NBLK, TW] in block order (m blocks then self)
        L = np.concatenate([le.reshape(NT, TW, M).transpose(0, 2, 1),
                            sl.reshape(NT, 1, TW)], axis=1)
        C = np.concatenate([ce.reshape(NT, TW, M).transpose(0, 2, 1),
                            sc.reshape(NT, 1, TW)], axis=1)
        # wrapped: idx position i -> partition i%16, column i//16
        locd = np.ascontiguousarray(
            L.reshape(NT, NBLK, NIC, 16).transpose(0, 3, 1, 2)
            .reshape(NT, 16, NBLK * NIC))
        chkd = np.ascontiguousarray(
            C.reshape(NT, NBLK, NIC, 16).transpose(0, 3, 1, 2)
            .reshape(NT, 16, NBLK * NIC))

        m = {"ashard": ash, "nbrT": nbrT, "locd": locd, "chkd": chkd}
        m.update(shared)
        in_maps.append(m)
    return in_maps


def kernel(atom_fea, nbr_fea, nbr_idx, W_full, b_full,
           bn1_gamma, bn1_beta, bn2_gamma, bn2_beta):
    atom_fea = np.asarray(atom_fea, np.float32)
    in_maps = _prep_in_maps(atom_fea, nbr_fea, nbr_idx, W_full,
                            bn1_gamma, bn1_beta, bn2_gamma, bn2_beta)
    nc = _get_nc()
    res = run_bass_kernel_spmd(nc, in_maps, list(range(NCORES)))
    out = np.empty((N, F), np.float32)
    for c in range(NCORES):
        d = res.results[c]["delta"].astype(np.float32)          # [64, NLOC]
        out[c * NLOC:(c + 1) * NLOC] = d.T
    out += atom_fea
    return out


# revision 50
# speedup vs baseline: 1.4852x; 1.2336x over previous
"""CGCNNConv on 8 TRN2 NeuronCores — transfer-optimized.

The axon tunnel (~63 MB/s effective) dominates wall time, so the design
minimizes host<->device bytes:
  - nbr_fea ships as fp8(e4m3), feature-major [NT, M, 64, TW] (9.8MB/core).
  - atom features ship SHARDED: each core sends its [12501, 128] bf16
    zero-padded slice (cols 64:128 zero, last row zero); a device AllGather
    builds the full [100008, 128] bf16 table once.  Row of atom j is
    r = j + j//12500 (one zero row after each shard); chunk c = r//25002
    gives 4 int16-addressable gather chunks, each with zero rows at local
    12500 and 25001 (used as the "no contribution" target).
  - neighbor gathers run ON DEVICE via gpsimd dma_gather (transpose mode,
    256B rows -> features land partition-major), 4 chunk-gathers per tile
    (6656 idxs = 12*512 edge slots + 512 self slots), summed with 3 DVE adds.
  - output is just delta = softplus(bn2(msg)) as [64, NPAD] bf16; the final
    atom_fea + delta add happens on host in f32.
Device per core, single NEFF:
  pass1: per t: 4 dma_gathers + adds -> gathered/self features; per (t,m):
         3 K=64 matmuls (self, gathered, nbr) -> PSUM [128, 512];
         DVE copies y->bf16 (+sum(y)), ACT accumulates sum(y^2),
         y stored to DRAM scratch [128, 153600] bf16.
  AllReduce [128, 2] -> BN1 scale/shift (linear bias b folds out).
  pass2: reload y halves stacked, ACT sigmoid/softplus, DVE -> msg [64, NPAD].
  AllReduce [64, 2] -> BN2; pass3: softplus(bn2(msg)) -> bf16 delta out.
"""
import os
import sys
sys.path.insert(0, '/opt/trn_rl_repo')
import numpy as np
import ml_dtypes

from concourse import bass, mybir
from concourse import library_config
from concourse.tile import TileContext
from concourse.bass_utils import run_bass_kernel_spmd

NCORES = 8
N = 100000
M = 12
F = 64                      # atom/nbr feature len
OUT = 128                   # 2F
NLOC = 12500                # atoms per core
NPAD = 12800                # padded atoms per core
NT = 25                     # tiles of 512 atoms
TW = 512                    # tile width (atoms)
NQ = NT * M                 # 300 (t, m) chunks
E = NQ * TW                 # 153600 edge slots per core
EPS = 1e-5
BN1_CNT = float(N * M)
BN2_CNT = float(N)

CH = 4                      # gather chunks
CHROWS = 25002              # table rows per chunk
TROWS = NCORES * (NLOC + 1)  # 100008 table rows
NBLK = M + 1                # gather blocks per tile: 12 edge slots + 1 self
NI = TW                     # idxs per dma_gather call (HW limit: 512 works)
NIC = NI // 16              # 32 idx columns per call

AF = mybir.ActivationFunctionType
f32 = mybir.dt.float32
bf16 = mybir.dt.bfloat16
fp8 = mybir.dt.float8e4
i16 = mybir.dt.int16

bf16np = ml_dtypes.bfloat16
fp8np = ml_dtypes.float8_e4m3


def _split_wait_lists(nc, limit=1):
    """This walrus codegen accepts at most one sync wait per instruction on
    several ISA structs; move excess waits onto preceding same-engine NoOp
    carriers (sequential same-engine execution keeps the conjunction)."""
    for bbname, bbw in nc.bb_map.items():
        il = bbw.bb.instructions
        i = 0
        while i < len(il):
            inst = il[i]
            if type(inst).__name__ in ('InstDMAGatherAnt', 'InstLdweights',
                                       'InstMatmult'):
                pass  # these ISA structs take a single wait: split below
            elif inst.engine in (mybir.EngineType.Pool, mybir.EngineType.PE) or \
                    type(inst).__name__ in ('InstISA',):
                i += 1
                continue
            si = getattr(inst, 'sync_info', None)
            waits = list(si.on_wait) if si is not None and si.on_wait is not None else []
            if len(waits) > limit:
                extra, keep = waits[:-limit], waits[-limit:]
                pos = i
                # don't break fused pairs (LdWeights+Matmult): insert before
                # the paired loader
                while pos > 0 and type(il[pos - 1]).__name__ in (
                        'InstLdweights', 'InstTensorLoad'):
                    pos -= 1
                for j in range(0, len(extra), limit):
                    nd = mybir.InstDrain(
                        name=nc.get_next_instruction_name(), ins=[], outs=[])
                    nd.engine = inst.engine
                    nd.sync_info = mybir.SyncInfo(
                        on_wait=list(extra[j:j + limit]), on_update=[])
                    il.insert(pos, nd)
                    pos += 1
                    i += 1
                inst.sync_info = mybir.SyncInfo(
                    on_wait=list(keep), on_update=list(si.on_update))
            i += 1


def _build():
    PH = os.environ.get("KPHASES", "123")
    nc = bass.Bass(num_devices=NCORES, num_swdge_queues=4)

    nbrT = nc.declare_dram_parameter("nbrT", [NT, M, F, TW], fp8, isOutput=False)
    ashard = nc.declare_dram_parameter("ashard", [NLOC + 1, F], bf16, isOutput=False)
    locd = nc.declare_dram_parameter("locd", [NT, 16, NBLK * NIC], i16,
                                     isOutput=False)
    chkd = nc.declare_dram_parameter("chkd", [NT, 16, NBLK * NIC], mybir.dt.uint8,
                                     isOutput=False)

    w_self = nc.declare_dram_parameter("w_self", [F, OUT], bf16, isOutput=False)
    w_g = nc.declare_dram_parameter("w_g", [F, OUT], bf16, isOutput=False)
    w_nb = nc.declare_dram_parameter("w_nb", [F, OUT], bf16, isOutput=False)
    gam1 = nc.declare_dram_parameter("gam1", [OUT, 1], f32, isOutput=False)
    bet1 = nc.declare_dram_parameter("bet1", [OUT, 1], f32, isOutput=False)
    gam2 = nc.declare_dram_parameter("gam2", [F, 1], f32, isOutput=False)
    bet2 = nc.declare_dram_parameter("bet2", [F, 1], f32, isOutput=False)
    delta = nc.declare_dram_parameter("delta", [F, NLOC], bf16, isOutput=True)

    ag_loc = nc.dram_tensor("ag_loc", [NLOC + 1, 128], bf16)
    ag_sh = nc.dram_tensor("ag_sh", [TROWS, 128], bf16, addr_space="Shared")
    y_dram = nc.dram_tensor("y_dram", [128, E], bf16)
    msg_dram = nc.dram_tensor("msg_dram", [F, NPAD], f32)
    st1_loc = nc.dram_tensor("st1_loc", [OUT, 2], f32)
    st1_sh = nc.dram_tensor("st1_sh", [OUT, 2], f32, addr_space="Shared")
    st2_loc = nc.dram_tensor("st2_loc", [F, 2], f32)
    st2_sh = nc.dram_tensor("st2_sh", [F, 2], f32, addr_space="Shared")

    groups = [list(range(NCORES))]

    with TileContext(nc, num_cores=NCORES) as tc:
        with tc.tile_pool(name="const", bufs=1) as cpool, \
             tc.tile_pool(name="work", bufs=4) as pool, \
             tc.tile_pool(name="p23", bufs=2) as pool2, \
             tc.tile_pool(name="idxp", bufs=2) as ipool, \
             tc.tile_pool(name="gat", bufs=6) as gpool, \
             tc.tile_pool(name="acc", bufs=1) as apool, \
             tc.tile_pool(name="psum", bufs=3, space="PSUM") as pp:

            nc.gpsimd.load_library(library_config.mlp)
            nireg = nc.gpsimd.to_reg(NI)

            # --- build the gather table: shard -> AllGather ---
            # spread [12501, 64] shard into rows of the 128-wide (256B) table
            # layout; cols 64:128 stay uninitialized (gather partitions 64:128
            # are never consumed)
            nc.sync.dma_start(out=ag_loc[:, 0:F], in_=ashard[:])
            nc.gpsimd.collective_compute(
                "AllGather", mybir.AluOpType.bypass, replica_groups=groups,
                ins=[ag_loc[:]], outs=[ag_sh[:]])

            # --- constants ---
            ws_sb = cpool.tile([F, OUT], bf16)
            nc.sync.dma_start(out=ws_sb[:], in_=w_self[:])
            wg_sb = cpool.tile([F, OUT], bf16)
            nc.sync.dma_start(out=wg_sb[:], in_=w_g[:])
            wn_sb = cpool.tile([F, OUT], bf16)
            nc.sync.dma_start(out=wn_sb[:], in_=w_nb[:])
            g1_sb = cpool.tile([OUT, 1], f32)
            nc.sync.dma_start(out=g1_sb[:], in_=gam1[:])
            b1_sb = cpool.tile([OUT, 1], f32)
            nc.sync.dma_start(out=b1_sb[:], in_=bet1[:])
            g2_sb = cpool.tile([F, 1], f32)
            nc.sync.dma_start(out=g2_sb[:], in_=gam2[:])
            b2_sb = cpool.tile([F, 1], f32)
            nc.sync.dma_start(out=b2_sb[:], in_=bet2[:])

            ysum = apool.tile([128, NQ], f32)
            ysq = apool.tile([128, NQ], f32)
            msum = apool.tile([F, NT], f32)
            ysq2 = apool.tile([F, NT], f32)

            # --- pass 1 ---
            def gather_block(t, blk, tag_a, tag_b):
                """4 chunk-gathers + tree adds -> [64, TW] bf16 in result[0:64].
                All 4 gathers issue back-to-back on separate SWDGE queues.
                Out-of-chunk / pad slots use idx 0 = the chunk-base zero row."""
                tiles = []
                for ch in range(CH):
                    T = gpool.tile([128, 1, NI], bf16,
                                   tag=tag_a if ch == 0 else tag_b)
                    nc.gpsimd.dma_gather(
                        T[:], ag_sh[CHROWS * ch:CHROWS * (ch + 1), :],
                        idx_sbs[t][ch][:, blk * NIC:(blk + 1) * NIC],
                        num_idxs=NI, num_idxs_reg=nireg, elem_size=128,
                        transpose=True, queue_num=ch)
                    tiles.append(T)
                A, B1, B2, B3 = tiles
                nc.vector.tensor_add(A[0:64, 0, :], A[0:64, 0, :], B1[0:64, 0, :])
                nc.vector.tensor_add(B2[0:64, 0, :], B2[0:64, 0, :], B3[0:64, 0, :])
                nc.vector.tensor_add(A[0:64, 0, :], A[0:64, 0, :], B2[0:64, 0, :])
                return A

            idx_sbs = {}
            NOGATH = os.environ.get("KNOGATHER", "0") == "1"
            NW = NBLK * NIC
            for t in range(int(os.environ.get("KNT1", NT))):
                if not NOGATH:
                    # expand compressed (local, chunk) -> 4 per-chunk idx
                    # tiles: idx = (chunk == ch) * local -> local in its
                    # chunk, 0 (= the chunk-base zero row) elsewhere.
                    locr = ipool.tile([128, NW], i16, tag="locr")
                    chkr = ipool.tile([128, NW], mybir.dt.uint8, tag="chkr")
                    for g in range(8):
                        nc.sync.dma_start(out=locr[16 * g:16 * (g + 1), :],
                                          in_=locd[t])
                        nc.scalar.dma_start(out=chkr[16 * g:16 * (g + 1), :],
                                            in_=chkd[t])
                    locf = ipool.tile([128, NW], f32, tag="locf")
                    nc.vector.tensor_copy(out=locf[:], in_=locr[:])
                    chkf = ipool.tile([128, NW], f32, tag="chkf")
                    nc.vector.tensor_copy(out=chkf[:], in_=chkr[:])
                    idxt = []
                    for ch in range(CH):
                        eq = ipool.tile([128, NW], f32, tag="eq")
                        nc.vector.tensor_scalar(
                            out=eq[:], in0=chkf[:], scalar1=float(ch),
                            scalar2=None, op0=mybir.AluOpType.is_equal)
                        nc.vector.tensor_mul(eq[:], eq[:], locf[:])
                        ic = ipool.tile([128, NW], i16, tag=f"ic{ch}")
                        nc.vector.tensor_copy(out=ic[:], in_=eq[:])
                        idxt.append(ic)
                    idx_sbs[t] = idxt
                    at = gather_block(t, M, "sA", "sB")
                for m in range(M):
                    q = t * M + m
                    if not NOGATH:
                        Am = gather_block(t, m, "gA", "gB")
                    nb8 = pool.tile([F, TW], fp8, tag="nb8")
                    nc.sync.dma_start(out=nb8[:], in_=nbrT[t, m])
                    nbv = pool.tile([F, TW], bf16, tag="nbv")
                    nc.vector.tensor_copy(out=nbv[:], in_=nb8[:])
                    yp = pp.tile([128, TW], f32, tag="yp")
                    if not NOGATH:
                        nc.tensor.matmul(yp[:], lhsT=ws_sb[:],
                                         rhs=at[0:64, 0, :],
                                         start=True, stop=False)
                        nc.tensor.matmul(yp[:], lhsT=wg_sb[:],
                                         rhs=Am[0:64, 0, :],
                                         start=False, stop=False)
                        nc.tensor.matmul(yp[:], lhsT=wn_sb[:], rhs=nbv[:],
                                         start=False, stop=True)
                    else:
                        nc.tensor.matmul(yp[:], lhsT=wn_sb[:], rhs=nbv[:],
                                         start=True, stop=True)
                    # DVE: psum -> bf16 sbuf copy, fused sum(y) accumulation
                    y_sb = pool.tile([128, TW], bf16, tag="ysb")
                    nc.vector.tensor_scalar(out=y_sb[:], in0=yp[:], scalar1=1.0,
                                            scalar2=0.0, op0=mybir.AluOpType.mult,
                                            op1=mybir.AluOpType.add,
                                            accum_out=ysum[:, q:q + 1])
                    # ACT: sum(y^2) from the bf16 copy
                    sqt = pool.tile([128, TW], f32, tag="sqt")
                    nc.scalar.activation(out=sqt[:], in_=y_sb[:], func=AF.Square,
                                         accum_out=ysq[:, q:q + 1])
                    nc.scalar.dma_start(out=y_dram[:, q * TW:(q + 1) * TW], in_=y_sb[:])

            # --- BN1 stats: reduce + allreduce ---
            st1 = apool.tile([OUT, 2], f32)
            nc.vector.tensor_reduce(st1[:, 0:1], ysum[:], axis=mybir.AxisListType.X,
                                    op=mybir.AluOpType.add)
            nc.vector.tensor_reduce(st1[:, 1:2], ysq[:], axis=mybir.AxisListType.X,
                                    op=mybir.AluOpType.add)
            nc.sync.dma_start(out=st1_loc[:], in_=st1[:])
            nc.gpsimd.collective_compute(
                "AllReduce", mybir.AluOpType.add, replica_groups=groups,
                ins=[st1_loc[:]], outs=[st1_sh[:]])
            st1g = apool.tile([OUT, 2], f32)
            nc.sync.dma_start(out=st1g[:], in_=st1_sh[:])

            # s1 = gam1 / sqrt(var + eps); t1 = bet1 - mean * s1
            mu1 = apool.tile([OUT, 1], f32)
            nc.vector.tensor_scalar_mul(mu1[:], st1g[:, 0:1], 1.0 / BN1_CNT)
            var1 = apool.tile([OUT, 1], f32)
            nc.vector.tensor_scalar_mul(var1[:], st1g[:, 1:2], 1.0 / BN1_CNT)
            musq = apool.tile([OUT, 1], f32)
            nc.vector.tensor_mul(musq[:], mu1[:], mu1[:])
            nc.vector.tensor_sub(var1[:], var1[:], musq[:])
            nc.vector.tensor_scalar_add(var1[:], var1[:], EPS)
            lnv1 = apool.tile([OUT, 1], f32)
            nc.scalar.activation(out=lnv1[:], in_=var1[:], func=AF.Ln)
            nc.vector.tensor_scalar_mul(lnv1[:], lnv1[:], -0.5)
            inv1 = apool.tile([OUT, 1], f32)
            nc.scalar.activation(out=inv1[:], in_=lnv1[:], func=AF.Exp)
            s1 = apool.tile([OUT, 1], f32)
            nc.vector.tensor_mul(s1[:], g1_sb[:], inv1[:])
            t1 = apool.tile([OUT, 1], f32)
            nc.vector.tensor_mul(t1[:], mu1[:], s1[:])
            nc.vector.tensor_sub(t1[:], b1_sb[:], t1[:])

            # stacked scale/shift: [s_f; s_f], [t_f; t_f], [s_c; s_c], [t_c; t_c]
            sf2 = apool.tile([128, 1], f32)
            tf2 = apool.tile([128, 1], f32)
            sc2 = apool.tile([128, 1], f32)
            tc2 = apool.tile([128, 1], f32)
            nc.vector.tensor_copy(out=sf2[0:64, :], in_=s1[0:64, :])
            nc.vector.tensor_copy(out=tf2[0:64, :], in_=t1[0:64, :])
            nc.sync.dma_start(out=sf2[64:128, :], in_=s1[0:64, :])
            nc.sync.dma_start(out=tf2[64:128, :], in_=t1[0:64, :])
            nc.sync.dma_start(out=sc2[0:64, :], in_=s1[64:128, :])
            nc.sync.dma_start(out=tc2[0:64, :], in_=t1[64:128, :])
            nc.sync.dma_start(out=sc2[64:128, :], in_=s1[64:128, :])
            nc.sync.dma_start(out=tc2[64:128, :], in_=t1[64:128, :])

            # --- pass 2: msg = sum_m sigmoid(f)*softplus(c) ---
            for t in range(NT if "2" in PH else 0):
                macc = pool2.tile([128, TW], f32, tag="macc")
                for k in range(M // 2):
                    q0 = (t * M + 2 * k) * TW
                    q1 = (t * M + 2 * k + 1) * TW
                    yf2 = pool2.tile([128, TW], bf16, tag="yf2")
                    nc.sync.dma_start(out=yf2[0:64, :], in_=y_dram[0:64, q0:q0 + TW])
                    nc.sync.dma_start(out=yf2[64:128, :], in_=y_dram[0:64, q1:q1 + TW])
                    yc2 = pool2.tile([128, TW], bf16, tag="yc2")
                    nc.scalar.dma_start(out=yc2[0:64, :], in_=y_dram[64:128, q0:q0 + TW])
                    nc.scalar.dma_start(out=yc2[64:128, :], in_=y_dram[64:128, q1:q1 + TW])
                    sg = pool2.tile([128, TW], f32, tag="sg")
                    nc.scalar.activation(out=sg[:], in_=yf2[:], func=AF.Sigmoid,
                                         bias=tf2[:, 0:1], scale=sf2[:, 0:1])
                    ec = pool2.tile([128, TW], f32, tag="ec")
                    nc.scalar.activation(out=ec[:], in_=yc2[:], func=AF.Exp,
                                         bias=tc2[:, 0:1], scale=sc2[:, 0:1])
                    sp = pool2.tile([128, TW], f32, tag="sp")
                    nc.scalar.activation(out=sp[:], in_=ec[:], func=AF.Ln, bias=1.0)
                    if k == 0:
                        nc.vector.tensor_mul(macc[:], sg[:], sp[:])
                    else:
                        prod = pool2.tile([128, TW], f32, tag="prod")
                        nc.vector.tensor_mul(prod[:], sg[:], sp[:])
                        nc.vector.tensor_add(macc[:], macc[:], prod[:])
                # fold top (m even) + bottom (m odd): shift bottom to partitions 0-63
                mlo = pool2.tile([64, TW], f32, tag="mlo")
                nc.sync.dma_start(out=mlo[:], in_=macc[64:128, :])
                mout = pool2.tile([64, TW], f32, tag="mout")
                nc.vector.tensor_add(mout[:], macc[0:64, :], mlo[:])
                if t == NT - 1:
                    nc.vector.memset(mout[:, NLOC - t * TW:TW], 0.0)
                # per-tile BN2 stat accumulation + spill msg tile to DRAM
                nc.vector.tensor_reduce(msum[:, t:t + 1], mout[:],
                                        axis=mybir.AxisListType.X,
                                        op=mybir.AluOpType.add)
                sq2t = pool2.tile([F, TW], f32, tag="sq2t")
                nc.scalar.activation(out=sq2t[:], in_=mout[:], func=AF.Square,
                                     accum_out=ysq2[:, t:t + 1])
                nc.sync.dma_start(out=msg_dram[:, t * TW:(t + 1) * TW], in_=mout[:])

            if "2" not in PH:
                nc.vector.memset(msum[:], 1.0)
                nc.vector.memset(ysq2[:], 1.0)
            st2 = apool.tile([F, 2], f32)
            nc.vector.tensor_reduce(st2[:, 0:1], msum[:], axis=mybir.AxisListType.X,
                                    op=mybir.AluOpType.add)
            nc.vector.tensor_reduce(st2[:, 1:2], ysq2[:], axis=mybir.AxisListType.X,
                                    op=mybir.AluOpType.add)
            nc.sync.dma_start(out=st2_loc[:], in_=st2[:])
            nc.gpsimd.collective_compute(
                "AllReduce", mybir.AluOpType.add, replica_groups=groups,
                ins=[st2_loc[:]], outs=[st2_sh[:]])
            st2g = apool.tile([F, 2], f32)
            nc.sync.dma_start(out=st2g[:], in_=st2_sh[:])

            mu2 = apool.tile([F, 1], f32)
            nc.vector.tensor_scalar_mul(mu2[:], st2g[:, 0:1], 1.0 / BN2_CNT)
            var2 = apool.tile([F, 1], f32)
            nc.vector.tensor_scalar_mul(var2[:], st2g[:, 1:2], 1.0 / BN2_CNT)
            msq2 = apool.tile([F, 1], f32)
            nc.vector.tensor_mul(msq2[:], mu2[:], mu2[:])
            nc.vector.tensor_sub(var2[:], var2[:], msq2[:])
            nc.vector.tensor_scalar_add(var2[:], var2[:], EPS)
            lnv2 = apool.tile([F, 1], f32)
            nc.scalar.activation(out=lnv2[:], in_=var2[:], func=AF.Ln)
            nc.vector.tensor_scalar_mul(lnv2[:], lnv2[:], -0.5)
            inv2 = apool.tile([F, 1], f32)
            nc.scalar.activation(out=inv2[:], in_=lnv2[:], func=AF.Exp)
            s2 = apool.tile([F, 1], f32)
            nc.vector.tensor_mul(s2[:], g2_sb[:], inv2[:])
            t2 = apool.tile([F, 1], f32)
            nc.vector.tensor_mul(t2[:], mu2[:], s2[:])
            nc.vector.tensor_sub(t2[:], b2_sb[:], t2[:])

            # --- pass 3: delta = softplus(bn2(msg)) -> bf16 ---
            for t in range(NT if "3" in PH else 1):
                mtin = pool2.tile([64, TW], f32, tag="mtin")
                nc.sync.dma_start(out=mtin[:], in_=msg_dram[:, t * TW:(t + 1) * TW])
                ex3 = pool2.tile([64, TW], f32, tag="ex3")
                nc.scalar.activation(out=ex3[:], in_=mtin[:], func=AF.Exp,
                                     bias=t2[:, 0:1], scale=s2[:, 0:1])
                d_sb = pool2.tile([64, TW], bf16, tag="dsb")
                nc.scalar.activation(out=d_sb[:], in_=ex3[:], func=AF.Ln, bias=1.0)
                w = min(TW, NLOC - t * TW)
                nc.sync.dma_start(out=delta[:, t * TW:t * TW + w], in_=d_sb[:, 0:w])

    _split_wait_lists(nc)
    mybir.codegen_inst_isa_subclasses(nc)
    return nc


_NC_CACHE = None


def _get_nc():
    global _NC_CACHE
    if _NC_CACHE is None:
        _NC_CACHE = _build()
    return _NC_CACHE


def _prep_in_maps(atom_fea, nbr_fea, nbr_idx, W_full,
                  bn1_gamma, bn1_beta, bn2_gamma, bn2_beta):
    atom_fea = np.asarray(atom_fea, np.float32)
    nbr_fea = np.asarray(nbr_fea, np.float32)
    nbr_idx = np.asarray(nbr_idx).astype(np.int64)
    W_full = np.asarray(W_full, np.float32)

    # global edge chunk/local indices: table row of atom j is
    # r = j + j//12500 + 1 (a zero row heads each shard; chunk bases are zero)
    r_all = (nbr_idx + nbr_idx // NLOC + 1).astype(np.int32)   # [N, M]
    c_all = r_all // CHROWS
    l_all = (r_all - c_all * CHROWS).astype(np.int16)

    shared = {
        "w_self": np.ascontiguousarray(W_full[0:64]).astype(bf16np),
        "w_g": np.ascontiguousarray(W_full[64:128]).astype(bf16np),
        "w_nb": np.ascontiguousarray(W_full[128:192]).astype(bf16np),
        "gam1": np.asarray(bn1_gamma, np.float32).reshape(OUT, 1).copy(),
        "bet1": np.asarray(bn1_beta, np.float32).reshape(OUT, 1).copy(),
        "gam2": np.asarray(bn2_gamma, np.float32).reshape(F, 1).copy(),
        "bet2": np.asarray(bn2_beta, np.float32).reshape(F, 1).copy(),
    }

    in_maps = []
    for c in range(NCORES):
        lo = c * NLOC
        # ashard: [12501, 64] bf16, ZERO row first (device spreads to 128-wide)
        ash = np.zeros((NLOC + 1, F), bf16np)
        ash[1:] = atom_fea[lo:lo + NLOC]

        # nbrT: [NT, M, F, TW] fp8, pad atoms zero
        nfs = np.zeros((NPAD, M, F), np.float32)
        nfs[:NLOC] = nbr_fea[lo:lo + NLOC]
        nbrT = np.ascontiguousarray(
            nfs.reshape(NT, TW, M, F).transpose(0, 2, 3, 1)).astype(fp8np)

        # compressed idx: per edge a chunk id (255 = none -> idx 0, the
        # chunk-base zero row) and a chunk-local row; device expands to
        # idx = (chunk == ch) * local per chunk
        ce = np.full((NPAD, M), 255, np.uint8)
        le = np.zeros((NPAD, M), np.int16)
        ce[:NLOC] = c_all[lo:lo + NLOC]
        le[:NLOC] = l_all[lo:lo + NLOC]

        # self slots: local atom a -> chunk c//2, local a+1 + 12501*(c%2);
        # pad atoms -> 0 (chunk-base zero row)
        a = np.arange(NPAD, dtype=np.int32)
        sl = np.where(a < NLOC, a + 1 + (NLOC + 1) * (c % 2), 0).astype(np.int16)
        sc = np.full(NPAD, c // 2, np.uint8)

        # [NT, NBLK, TW] in block order (m blocks then self)
        L = np.concatenate([le.reshape(NT, TW, M).transpose(0, 2, 1),
                            sl.reshape(NT, 1, TW)], axis=1)
        C = np.concatenate([ce.reshape(NT, TW, M).transpose(0, 2, 1),
                            sc.reshape(NT, 1, TW)], axis=1)
        # wrapped: idx position i -> partition i%16, column i//16
        locd = np.ascontiguousarray(
            L.reshape(NT, NBLK, NIC, 16).transpose(0, 3, 1, 2)
            .reshape(NT, 16, NBLK * NIC))
        chkd = np.ascontiguousarray(
            C.reshape(NT, NBLK, NIC, 16).transpose(0, 3, 1, 2)
            .reshape(NT, 16, NBLK * NIC))

        m = {"ashard": ash, "nbrT": nbrT, "locd": locd, "chkd": chkd}
        m.update(shared)
        in_maps.append(m)
    return in_maps


def kernel(atom_fea, nbr_fea, nbr_idx, W_full, b_full,
           bn1_gamma, bn1_beta, bn2_gamma, bn2_beta):
    atom_fea = np.asarray(atom_fea, np.float32)
    in_maps = _prep_in_maps(atom_fea, nbr_fea, nbr_idx, W_full,
                            bn1_gamma, bn1_beta, bn2_gamma, bn2_beta)
    nc = _get_nc()
    res = run_bass_kernel_spmd(nc, in_maps, list(range(NCORES)))
    out = np.empty((N, F), np.float32)
    for c in range(NCORES):
        d = res.results[c]["delta"].astype(np.float32)          # [64, NLOC]
        out[c * NLOC:(c + 1) * NLOC] = d.T
    out += atom_fea
    return out
